# revision 1
# baseline (speedup 1.0000x reference)
"""CapsuleRewardHead Trainium2 kernel (8-core data parallel).

Math (per batch row b):
    primary = x @ W + b_lin                    [B, 128]  (128 = 8 caps x 16 dim)
    u_hat[b,o,i,j] = sum_c primary[b,i,c] * out_caps[o,i,c,j]
    3 rounds of dynamic routing over N=32 capsule pairs (o,i), D=16
    out[b] = |squash(s_final)|

Device strategy per core (2048 batch rows):
  - host: quantize x shard to fp8 e4m3 and pre-tile to [sp][queue][128 part]
    [hc][b] so every DMA issue reads one fully contiguous DRAM block; W to
    fp8 scaled by 1024 (descaled via caps/capsum); W/caps pre-transposed so
    their loads are contiguous too. Linear bias rides as a K=1 bf16 matmul.
  - MM1 (PE): DoubleRow fp8 matmuls contract h-chunk PAIRS (256 rows/pass)
    into PSUM: primaryT[ic, b] per 512-col super.
  - MM2 (PE): u_hat[b, (o,i,j)] via block-diagonal capsule matrices straight
    into routing layout [128b, 512]; an extra N=16 matmul against
    sum_o(caps) yields round-0's uniform-coefficient sum t0 for free.
  - routing engine split: softmax exp emits compact [K,N] on ACT; ACT also
    materializes the e/t broadcasts so the big multiplies run in DVE 2x
    bf16 mode; round-0's agreement multiply goes to GPSIMD with a
    broadcast t read (GPSIMD is 1x anyway). The n/d reductions run as
    pairwise halving trees of 2x-mode tensor_tensor adds (tensor_reduce
    has no fast mode; trees are ~2x faster). sqrt via bit-trick seed;
    unnormalized accumulators (q = |t|^2, se = sum e) keep the per-round
    scalar chain short.
"""

import os

import numpy as np
import ml_dtypes

B = 16384
HIDDEN = 4096
NUM_OBJ = 4
NUM_CAPS = 8
CAP_DIM = 16
N_ROUTE = 32  # NUM_OBJ * NUM_CAPS
N_CORES = 8

LAST_EXEC_TIME_NS = None  # set after each run when BASS_TRACE=1

BF16 = ml_dtypes.bfloat16
FP8 = ml_dtypes.float8_e4m3
W_SCALE = 1024.0
SQRT_MAGIC = 0x1FBD1DF5
NQ = 4  # x sub-DMA issues per super (4KB descriptors)


def _ap(ap, dims):
    import concourse.bass as bass

    return bass.AP(tensor=ap.tensor, offset=ap.offset, ap=dims)


def build_bass(hidden=HIDDEN, b_sh=B // N_CORES, batch_plan=(8, 8)):
    import concourse.tile as tile
    from concourse import bacc, mybir

    NH = hidden // 128
    NCH = b_sh // 128  # 128-row chunks
    SUP = 512
    NSUP = b_sh // SUP
    CPS = SUP // 128
    HQ = NH // NQ
    assert sum(batch_plan) == NCH
    N, D = N_ROUTE, CAP_DIM
    dt = mybir.dt
    AX = mybir.AxisListType
    OP = mybir.AluOpType
    AF = mybir.ActivationFunctionType
    PM = mybir.MatmulPerfMode

    batches = []
    pos = 0
    for k in batch_plan:
        batches.append(list(range(pos, pos + k)))
        pos += k
    last_chunk_to_batch = {b[-1]: bi for bi, b in enumerate(batches)}
    chunk_to_batch = {}
    for bi, chs in enumerate(batches):
        for ch in chs:
            chunk_to_batch[ch] = bi

    nc = bacc.Bacc("TRN2", target_bir_lowering=False, debug=False, num_devices=N_CORES)

    xt_ap = nc.dram_tensor(
        "xt", [NSUP, NQ, 128, HQ, SUP], dt.float8e4, kind="ExternalInput"
    ).ap()
    w_ap = nc.dram_tensor("w", [128, NH, 128], dt.float8e4, kind="ExternalInput").ap()
    caps_ap = nc.dram_tensor(
        "caps", [128, NUM_OBJ, 128], dt.bfloat16, kind="ExternalInput"
    ).ap()
    capsum_ap = nc.dram_tensor(
        "capsum", [128, CAP_DIM], dt.bfloat16, kind="ExternalInput"
    ).ap()
    bias_ap = nc.dram_tensor("bias", [1, 256], dt.bfloat16, kind="ExternalInput").ap()
    out_ap = nc.dram_tensor("out", [b_sh], dt.float32, kind="ExternalOutput").ap()

    with tile.TileContext(nc) as tc:
        with (
            tc.tile_pool(name="singles", bufs=1) as singles,
            tc.tile_pool(name="xs", bufs=2) as xs_pool,
            tc.tile_pool(name="primt", bufs=2) as primt_pool,
            tc.tile_pool(name="batch", bufs=1) as bpool,
            tc.tile_pool(name="tmp", bufs=2) as tmp_pool,
            tc.tile_pool(name="sm", bufs=8) as sm_pool,
            tc.tile_pool(name="psum_p", bufs=2, space="PSUM") as psp_pool,
            tc.tile_pool(name="psum_u", bufs=3, space="PSUM") as psu_pool,
            tc.tile_pool(name="psum_t", bufs=2, space="PSUM") as pst_pool,
            tc.tile_pool(name="psum_w", bufs=1, space="PSUM") as psw_pool,
        ):
            w_sb = singles.tile([128, NH, 128], dt.float8e4)
            caps_sb = singles.tile([128, NUM_OBJ, 128], dt.bfloat16)
            capsum_sb = singles.tile([128, CAP_DIM], dt.bfloat16)
            bias_sb = singles.tile([1, 256], dt.bfloat16)

            def issue_params():
                # issued after super 0's x DMAs so x streams first
                nc.sync.dma_start(out=w_sb[:], in_=w_ap[:, :, :])
                nc.sync.dma_start(out=caps_sb[:], in_=caps_ap[:, :, :])
                nc.sync.dma_start(out=capsum_sb[:], in_=capsum_ap[:, :])
                nc.sync.dma_start(out=bias_sb[:], in_=bias_ap[:, :])
            magic_sb = singles.tile([128, 1], dt.uint32)
            nc.vector.memset(magic_sb[:], SQRT_MAGIC)
            out_sb = singles.tile([128, NCH], dt.float32)
            warm_sb = singles.tile([128, 2, SUP], dt.float8e4)
            nc.vector.memset(warm_sb.rearrange("p a b -> p (a b)"), 0)

            uh_all, t_all, b_all = {}, {}, {}
            for bi, chs in enumerate(batches):
                K = len(chs)
                # [N, K, D]: n outermost so n-halving tree operands are
                # fully contiguous 1D (hardware 2x mode needs that)
                uh_all[bi] = bpool.tile(
                    [128, N, K, D], dt.bfloat16, tag=f"uh{bi}", name=f"uh{bi}"
                )
                t_all[bi] = bpool.tile(
                    [128, K, D], dt.bfloat16, tag=f"t{bi}", name=f"t{bi}"
                )
                # two logit buffers: the r1 update writes out-of-place
                # (in-place DVE ops run ~4x slower), bf16 for 2x mode
                b_all[bi] = (
                    bpool.tile([128, K, N], dt.bfloat16, tag=f"b{bi}a",
                               name=f"b{bi}a"),
                    bpool.tile([128, K, N], dt.bfloat16, tag=f"b{bi}b",
                               name=f"b{bi}b"),
                )

            def smt(K, tag, dtype=dt.float32):
                return sm_pool.tile([128, K], dtype, tag=tag, name=tag)

            def sqrt_half(q, K):
                """bit-trick sqrt seed; error washes out through squash."""
                qu = q.bitcast(dt.uint32)
                s1 = smt(K, "sq1", dt.uint32)
                nc.vector.tensor_single_scalar(
                    s1[:], qu, 1, op=OP.logical_shift_right
                )
                s2 = smt(K, "sq2", dt.uint32)
                nc.vector.tensor_tensor(
                    s2[:],
                    s1[:],
                    _ap(magic_sb[:], [magic_sb[:].ap[0], [0, K]]),
                    op=OP.add,
                )
                return s2.bitcast(dt.float32)  # ~3.5% sqrt approx (validated)

            def tree_n(tag, src, K, dst):
                """[128,N,K,D] bf16 -> dst [128,K,D] fp32 via halving adds.

                Halves along outermost n are contiguous 1D blocks -> 2x mode.
                """
                cur = src
                w = N
                with nc.allow_low_precision(reason="tree bf16 validated"):
                    while w > 2:
                        w //= 2
                        nxt = tmp_pool.tile(
                            [128, w, K, D], dt.bfloat16, tag=f"{tag}{w}",
                            name=f"{tag}{w}",
                        )
                        nc.vector.tensor_tensor(
                            nxt[:], cur[:, 0:w, :, :], cur[:, w : 2 * w, :, :],
                            op=OP.add,
                        )
                        cur = nxt
                    nc.vector.tensor_tensor(
                        dst, cur[:, 0, :, :], cur[:, 1, :, :], op=OP.add
                    )

            def routing_batch(bi):
                chs = batches[bi]
                K = len(chs)
                uh = uh_all[bi]
                tt = t_all[bi]
                bA, bB = b_all[bi]
                for r in range(3):
                    if r > 0:
                        bcur = bA if r == 1 else bB
                        if r == 2:
                            # r2 logits can reach ~56; subtract the max so
                            # se^2 stays in fp32 range. r1 logits are <~33
                            # (se^2 < 7e30), so r1 exps directly.
                            mx = smt(K, "mx", dt.bfloat16)
                            with nc.allow_low_precision(reason="bf16 logits"):
                                nc.vector.tensor_reduce(
                                    mx[:], bcur[:], axis=AX.X, op=OP.max
                                )
                            bsub = sm_pool.tile(
                                [128, K, N], dt.bfloat16, tag="bsub", name="bsub"
                            )
                            nc.vector.tensor_tensor(
                                bsub[:],
                                bcur[:],
                                _ap(mx[:], [*mx[:].ap, [0, N]]),
                                op=OP.subtract,
                            )
                            esrc = bsub[:]
                        else:
                            esrc = bcur[:]
                        e = sm_pool.tile(
                            [128, K, N], dt.bfloat16, tag="esm", name="esm"
                        )
                        nc.scalar.activation(e[:], esrc, AF.Exp)
                        se = smt(K, "se")
                        nc.vector.tensor_reduce(se[:], e[:], axis=AX.X, op=OP.add)
                        erep = tmp_pool.tile(
                            [128, N, K, D], dt.bfloat16, tag="erep", name="erep"
                        )
                        e3 = e[:]
                        nc.scalar.copy(
                            erep[:], _ap(e3, [e3.ap[0], [1, N], [N, K], [0, D]])
                        )
                        wmul = tmp_pool.tile(
                            [128, N, K, D], dt.bfloat16, tag="wmul", name="wmul"
                        )
                        nc.vector.tensor_tensor(
                            wmul.rearrange("p n k d -> p (n k d)"),
                            uh.rearrange("p n k d -> p (n k d)"),
                            erep.rearrange("p n k d -> p (n k d)"),
                            op=OP.mult,
                        )
                        tree_n("tn", wmul, K, tt[:])
                    # q = |t|^2, den = se^2 + q, rden = 1/den
                    sq = sm_pool.tile(
                        [128, K, D], dt.bfloat16, tag="sqv", name="sqv"
                    )
                    nc.vector.tensor_tensor(sq[:], tt[:], tt[:], op=OP.mult)
                    q = smt(K, "q")
                    nc.vector.tensor_reduce(q[:], sq[:], axis=AX.X, op=OP.add)
                    den = smt(K, "den")
                    if r == 0:
                        nc.vector.tensor_single_scalar(
                            den[:], q[:], float(N * N), op=OP.add
                        )
                    else:
                        se2 = smt(K, "se2")
                        nc.vector.tensor_mul(se2[:], se[:], se[:])
                        nc.vector.tensor_add(den[:], q[:], se2[:])

                    rden = smt(K, "rden")
                    nc.vector.reciprocal(rden[:], den[:])
                    if r < 2:
                        sm = sqrt_half(q[:], K)
                        alpha2 = smt(K, "alpha2")
                        nc.vector.tensor_mul(alpha2[:], sm, rden[:])
                        amul = tmp_pool.tile(
                            [128, N, K, D], dt.bfloat16, tag="amul", name="amul"
                        )
                        tt3 = tt[:]
                        t_bc = _ap(tt3, [tt3.ap[0], [0, N], [D, K], [1, D]])
                        if r == 0:
                            # GPSIMD multiply with broadcast t read (1x there
                            # anyway); frees DVE and skips materializing trep.
                            # Split in two so the d-tree can start earlier.
                            KH = max(K // 2, 1)
                            nc.gpsimd.tensor_tensor(
                                amul[:, :, 0:KH, :],
                                uh[:, :, 0:KH, :],
                                _ap(tt3, [tt3.ap[0], [0, N], [D, KH], [1, D]]),
                                op=OP.mult,
                            )
                            if KH < K:
                                t3b = tt[:, KH:K, :]
                                nc.gpsimd.tensor_tensor(
                                    amul[:, :, KH:K, :],
                                    uh[:, :, KH:K, :],
                                    _ap(
                                        t3b,
                                        [t3b.ap[0], [0, N], [D, K - KH], [1, D]],
                                    ),
                                    op=OP.mult,
                                )
                        else:
                            trep = tmp_pool.tile(
                                [128, N, K, D], dt.bfloat16, tag="trep",
                                name="trep",
                            )
                            nc.scalar.copy(trep[:], t_bc)
                            nc.vector.tensor_tensor(
                                amul.rearrange("p n k d -> p (n k d)"),
                                uh.rearrange("p n k d -> p (n k d)"),
                                trep.rearrange("p n k d -> p (n k d)"),
                                op=OP.mult,
                            )
                        # d-reduction as a gapped halving tree (2x mode
                        # beats tensor_reduce's 1x)
                        dta = sm_pool.tile(
                            [128, N, K], dt.bfloat16, tag="dta", name="dta"
                        )
                        cur = amul
                        w = D
                        with nc.allow_low_precision(reason="bf16 validated"):
                            while w > 2:
                                w //= 2
                                nxt = tmp_pool.tile(
                                    [128, N, K, w], dt.bfloat16, tag=f"td{w}",
                                    name=f"td{w}",
                                )
                                nc.vector.tensor_tensor(
                                    nxt[:], cur[:, :, :, 0:w],
                                    cur[:, :, :, w : 2 * w], op=OP.add,
                                )
                                cur = nxt
                            nc.vector.tensor_tensor(
                                dta[:], cur[:, :, :, 0], cur[:, :, :, 1],
                                op=OP.add,
                            )
                        # dta is [n, k]; read it back transposed for the
                        # [k, n]-ordered logit update
                        d3 = dta[:]
                        dta_kn = _ap(d3, [d3.ap[0], [1, K], [K, N]])
                        a_bc = _ap(alpha2[:], [*alpha2[:].ap, [0, N]])
                        if r == 0:
                            nc.vector.tensor_tensor(
                                bA[:], dta_kn, a_bc, op=OP.mult
                            )
                        else:
                            badd = sm_pool.tile(
                                [128, K, N], dt.bfloat16, tag="badd", name="badd"
                            )
                            nc.vector.tensor_tensor(
                                badd[:], dta_kn, a_bc, op=OP.mult
                            )
                            with nc.allow_low_precision(reason="bf16 logits"):
                                nc.vector.tensor_tensor(
                                    bB[:], bA[:], badd[:], op=OP.add
                                )
                    else:
                        nc.vector.tensor_mul(
                            out_sb[:, chs[0] : chs[0] + K], q[:], rden[:]
                        )
                        nc.sync.dma_start(
                            out=out_ap.rearrange("(c p) -> p c", p=128)[
                                :, chs[0] : chs[0] + K
                            ],
                            in_=out_sb[:, chs[0] : chs[0] + K],
                        )

            # PE p-state warmup while super 0 streams in
            psw = psw_pool.tile([128, SUP], dt.float32)
            for wi in range(24):
                nc.tensor.matmul(
                    psw[:],
                    warm_sb[:, 0, 0:128],
                    warm_sb[:, 1, :],
                    start=(wi == 0),
                    stop=(wi == 23),
                )

            for sp in range(NSUP):
                xs = xs_pool.tile([128, NH, SUP], dt.float8e4)
                for qd in range(NQ):
                    nc.sync.dma_start(
                        out=xs[:, qd * HQ : (qd + 1) * HQ, :],
                        in_=xt_ap[sp, qd],
                    )
                if sp == 0:
                    issue_params()
                psp = psp_pool.tile([128, SUP], dt.float32)
                # Linear bias rides as a K=1 bf16 matmul against ones
                ones_bc = _ap(
                    bias_sb[:, 128:256],
                    [bias_sb[:, 128:256].ap[0], [0, CPS], [1, 128]],
                )
                nc.tensor.matmul(
                    psp[:], bias_sb[:, 0:128], ones_bc, start=True, stop=False
                )
                for hp in range(NH // 2):
                    nc.tensor.matmul(
                        psp[:],
                        w_sb[:, 2 * hp : 2 * hp + 2, :],
                        xs[:, 2 * hp : 2 * hp + 2, :],
                        start=False,
                        stop=(hp == NH // 2 - 1),
                        perf_mode=PM.DoubleRow,
                    )
                primt = primt_pool.tile([128, SUP], dt.bfloat16)
                nc.scalar.copy(primt[:], psp[:])

                for c in range(CPS):
                    s = sp * CPS + c
                    bi = chunk_to_batch[s]
                    k = s - batches[bi][0]
                    lhsT = primt[:, c * 128 : (c + 1) * 128]
                    psu = psu_pool.tile([128, NUM_OBJ * 128], dt.float32)
                    nc.tensor.matmul(
                        psu[:],
                        lhsT,
                        caps_sb.rearrange("p o f -> p (o f)"),
                        start=True,
                        stop=True,
                    )
                    pst = pst_pool.tile([128, CAP_DIM], dt.float32)
                    nc.tensor.matmul(
                        pst[:], lhsT, capsum_sb[:], start=True, stop=True
                    )
                    nc.scalar.copy(
                        uh_all[bi][:, :, k, :],
                        psu.rearrange("p (n d) -> p n d", n=N),
                    )
                    nc.scalar.copy(t_all[bi][:, k, :], pst[:])

                    if s in last_chunk_to_batch:
                        routing_batch(last_chunk_to_batch[s])

    nc.compile()
    return nc


def _prep_params(W, b_lin, out_caps, hidden=HIDDEN):
    NH = hidden // 128
    w_f = np.ascontiguousarray(
        (W.astype(np.float32) * W_SCALE)
        .reshape(NH, 128, NUM_CAPS * CAP_DIM)
        .transpose(1, 0, 2)
    ).astype(FP8)
    caps_bd = np.zeros((NUM_OBJ, 128, 128), np.float32)
    for o in range(NUM_OBJ):
        for i in range(NUM_CAPS):
            caps_bd[
                o, i * CAP_DIM : (i + 1) * CAP_DIM, i * CAP_DIM : (i + 1) * CAP_DIM
            ] = out_caps[o, i]
    caps_bd /= W_SCALE
    capsum = caps_bd.sum(0)
    caps_bd = np.ascontiguousarray(caps_bd.transpose(1, 0, 2)).astype(BF16)
    capsum_t0 = np.zeros((128, CAP_DIM), np.float32)
    for i in range(NUM_CAPS):
        capsum_t0[i * CAP_DIM : (i + 1) * CAP_DIM, :] = capsum[
            i * CAP_DIM : (i + 1) * CAP_DIM, i * CAP_DIM : (i + 1) * CAP_DIM
        ]
    bias_row = np.concatenate(
        [
            b_lin.astype(np.float32).reshape(1, 128) * W_SCALE,
            np.ones((1, 128), np.float32),
        ],
        axis=1,
    ).astype(BF16)
    return w_f, caps_bd, np.ascontiguousarray(capsum_t0).astype(BF16), bias_row


_NC_CACHE = {}


def kernel(x, W, b_lin, out_caps):
    global LAST_EXEC_TIME_NS
    from concourse.bass_utils import run_bass_kernel_spmd

    x = np.asarray(x)
    W = np.asarray(W)
    b_lin = np.asarray(b_lin)
    out_caps = np.asarray(out_caps)
    bsz, hidden = x.shape
    b_sh = bsz // N_CORES
    NH = hidden // 128
    SUP = 512
    NSUP = b_sh // SUP
    HQ = NH // NQ

    key = (hidden, b_sh)
    if key not in _NC_CACHE:
        _NC_CACHE[key] = build_bass(hidden=hidden, b_sh=b_sh)
    nc = _NC_CACHE[key]

    w_f, caps_bd, capsum_t0, bias_row = _prep_params(W, b_lin, out_caps, hidden)

    in_maps = []
    for i in range(N_CORES):
        shard = x[i * b_sh : (i + 1) * b_sh]
        # [sp, qd, p, hcq, b]: every DMA issue reads contiguous DRAM
        xt = np.ascontiguousarray(
            shard.reshape(NSUP, SUP, NQ, HQ, 128).transpose(0, 2, 4, 3, 1)
        ).astype(FP8)
        in_maps.append(
            {
                "xt": xt,
                "w": w_f,
                "caps": caps_bd,
                "capsum": capsum_t0,
                "bias": bias_row,
            }
        )

    res = run_bass_kernel_spmd(
        nc,
        in_maps,
        core_ids=list(range(N_CORES)),
        trace=bool(int(os.environ.get("BASS_TRACE", "0") or "0")),
    )
    LAST_EXEC_TIME_NS = res.exec_time_ns
    return np.concatenate([res.results[i]["out"] for i in range(N_CORES)])



# revision 7
# speedup vs baseline: 1.2691x; 1.2691x over previous
"""CapsuleRewardHead Trainium2 kernel (8-core data parallel), v2.

Math (per batch row b):
    primary = x @ W + b_lin                    [B, 128]  (128 = 8 caps x 16 dim)
    u_hat[b,o,i,j] = sum_c primary[b,i,c] * out_caps[o,i,c,j]
    3 rounds of dynamic routing over N=32 capsule pairs (o,i), D=16
    out[b] = |squash(s_final)|

Device strategy per core (2048 batch rows):
  - host: quantize x shard to fp8 e4m3, laid out [sp][128 part][hp][b] so each
    super is ONE contiguous 2MB DMA (16KB/partition) -> ~6us super latency,
    full 16-SDMA-engine spread, pipelined with MM1.
  - MM1 (PE): DoubleRow fp8 matmuls contract h-chunk pairs into PSUM:
    primaryT[ic, b] per 512-col super; linear bias rides as a K=1 bf16 matmul.
  - MM2 (PE): per 128-row chunk, TWO matmuls against differently-ordered
    block-diagonal caps constants give u_hat in both [K,N,D] (d-inner) and
    [K,D,N] (n-inner) layouts, plus a capsum matmul for round-0's t0.
  - routing: all elementwise on DVE with DIRECT broadcast reads (inner-step-1
    APs hit 2x mode on HW; verified by microbench — no erep/trep
    materialization, no GPSIMD which contends with DVE for the SBUF port).
    n-trees run on the n-inner copy, d-trees on the d-inner copy so every
    tree level is a 2x-mode halving add and the agreement lands directly in
    the [K,N] logit layout. sqrt via bit-trick seed; unnormalized
    accumulators (q = |t|^2, se = sum e).
  - emission order interleaves MM2 chunk blocks with group-0 rounds so ACT
    psum->sbuf copies never queue behind chain-critical exps.
"""

import os

import numpy as np
import ml_dtypes

B = 16384
HIDDEN = 4096
NUM_OBJ = 4
NUM_CAPS = 8
CAP_DIM = 16
N_ROUTE = 32  # NUM_OBJ * NUM_CAPS
N_CORES = 8

LAST_EXEC_TIME_NS = None  # set after each run when BASS_TRACE=1

BF16 = ml_dtypes.bfloat16
FP8 = ml_dtypes.float8_e4m3
W_SCALE = 1024.0
SQRT_MAGIC = 0x1FBD1DF5


def _ap(ap, dims):
    import concourse.bass as bass

    return bass.AP(tensor=ap.tensor, offset=ap.offset, ap=dims)


def build_bass(hidden=HIDDEN, b_sh=B // N_CORES, batch_plan=(8, 8)):
    import concourse.tile as tile
    from concourse import bacc, mybir

    NH = hidden // 128
    NCH = b_sh // 128  # 128-row chunks
    SUP = 512
    NSUP = b_sh // SUP
    CPS = SUP // 128
    assert sum(batch_plan) == NCH
    N, D = N_ROUTE, CAP_DIM
    dt = mybir.dt
    AX = mybir.AxisListType
    OP = mybir.AluOpType
    AF = mybir.ActivationFunctionType
    PM = mybir.MatmulPerfMode

    batches = []
    pos = 0
    for k in batch_plan:
        batches.append(list(range(pos, pos + k)))
        pos += k
    chunk_to_batch = {}
    for bi, chs in enumerate(batches):
        for ch in chs:
            chunk_to_batch[ch] = bi

    nc = bacc.Bacc("TRN2", target_bir_lowering=False, debug=False, num_devices=N_CORES)

    xt_ap = nc.dram_tensor(
        "xt", [NSUP, 128, NH, SUP], dt.float8e4, kind="ExternalInput"
    ).ap()
    w_ap = nc.dram_tensor("w", [128, NH, 128], dt.float8e4, kind="ExternalInput").ap()
    capsd_ap = nc.dram_tensor(
        "capsd", [128, N * D], dt.bfloat16, kind="ExternalInput"
    ).ap()
    capsn_ap = nc.dram_tensor(
        "capsn", [128, D * N], dt.bfloat16, kind="ExternalInput"
    ).ap()
    capsum_ap = nc.dram_tensor(
        "capsum", [128, CAP_DIM], dt.bfloat16, kind="ExternalInput"
    ).ap()
    bias_ap = nc.dram_tensor("bias", [1, 256], dt.bfloat16, kind="ExternalInput").ap()
    out_ap = nc.dram_tensor("out", [b_sh], dt.float32, kind="ExternalOutput").ap()

    with tile.TileContext(nc) as tc:
        with (
            tc.tile_pool(name="singles", bufs=1) as singles,
            tc.tile_pool(name="xs", bufs=2) as xs_pool,
            tc.tile_pool(name="primt", bufs=2) as primt_pool,
            tc.tile_pool(name="batch", bufs=1) as bpool,
            tc.tile_pool(name="tmp", bufs=2) as tmp_pool,
            tc.tile_pool(name="sm", bufs=8) as sm_pool,
            tc.tile_pool(name="psum_p", bufs=2, space="PSUM") as psp_pool,
            tc.tile_pool(name="psum_u", bufs=2, space="PSUM") as psu_pool,
            tc.tile_pool(name="psum_t", bufs=1, space="PSUM") as pst_pool,
            tc.tile_pool(name="psum_w", bufs=1, space="PSUM") as psw_pool,
        ):
            w_sb = singles.tile([128, NH, 128], dt.float8e4)
            capsd_sb = singles.tile([128, N * D], dt.bfloat16)
            capsn_sb = singles.tile([128, D * N], dt.bfloat16)
            capsum_sb = singles.tile([128, CAP_DIM], dt.bfloat16)
            bias_sb = singles.tile([1, 256], dt.bfloat16)

            def issue_params():
                # issued after super 0's x DMA so x streams first
                nc.sync.dma_start(out=w_sb[:], in_=w_ap[:, :, :])
                nc.sync.dma_start(out=capsd_sb[:], in_=capsd_ap[:, :])
                nc.sync.dma_start(out=capsn_sb[:], in_=capsn_ap[:, :])
                nc.sync.dma_start(out=capsum_sb[:], in_=capsum_ap[:, :])
                nc.sync.dma_start(out=bias_sb[:], in_=bias_ap[:, :])

            magic_sb = singles.tile([128, 1], dt.uint32)
            nc.vector.memset(magic_sb[:], SQRT_MAGIC)
            out_sb = singles.tile([128, NCH], dt.float32)
            warm_sb = singles.tile([128, 2, SUP], dt.float8e4)
            nc.vector.memset(warm_sb.rearrange("p a b -> p (a b)"), 0)

            uhd_all, uhn_all, t_all, b_all = {}, {}, {}, {}
            for bi, chs in enumerate(batches):
                K = len(chs)
                uhd_all[bi] = bpool.tile(
                    [128, K, N, D], dt.bfloat16, tag=f"uhd{bi}", name=f"uhd{bi}"
                )
                uhn_all[bi] = bpool.tile(
                    [128, K, D, N], dt.bfloat16, tag=f"uhn{bi}", name=f"uhn{bi}"
                )
                t_all[bi] = bpool.tile(
                    [128, K, D], dt.bfloat16, tag=f"t{bi}", name=f"t{bi}"
                )
                # two logit buffers: the r1 update writes out-of-place
                # (in-place DVE ops run ~4x slower), bf16 for 2x mode
                b_all[bi] = (
                    bpool.tile([128, K, N], dt.bfloat16, tag=f"b{bi}a",
                               name=f"b{bi}a"),
                    bpool.tile([128, K, N], dt.bfloat16, tag=f"b{bi}b",
                               name=f"b{bi}b"),
                )

            def smt(K, tag, dtype=dt.float32):
                return sm_pool.tile([128, K], dtype, tag=tag, name=tag)

            def sqrt_half(q, K):
                """bit-trick sqrt seed; error washes out through squash."""
                qu = q.bitcast(dt.uint32)
                s1 = smt(K, "sq1", dt.uint32)
                nc.vector.tensor_single_scalar(
                    s1[:], qu, 1, op=OP.logical_shift_right
                )
                s2 = smt(K, "sq2", dt.uint32)
                nc.vector.tensor_tensor(
                    s2[:],
                    s1[:],
                    _ap(magic_sb[:], [magic_sb[:].ap[0], [0, K]]),
                    op=OP.add,
                )
                return s2.bitcast(dt.float32)  # ~3.5% sqrt approx (validated)

            def tree_n(src, K, dst):
                """wm [128,K,D,N] bf16 -> dst t [128,K,D] via halving adds on
                innermost n (every level inner step 1 -> 2x mode)."""
                cur = src
                w = N
                with nc.allow_low_precision(reason="tree bf16 validated"):
                    while w > 2:
                        w //= 2
                        nxt = tmp_pool.tile(
                            [128, K, D, w], dt.bfloat16, tag=f"tn{w}",
                            name=f"tn{w}",
                        )
                        nc.vector.tensor_tensor(
                            nxt[:], cur[:, :, :, 0:w], cur[:, :, :, w : 2 * w],
                            op=OP.add,
                        )
                        cur = nxt
                    nc.vector.tensor_tensor(
                        dst, cur[:, :, :, 0], cur[:, :, :, 1], op=OP.add
                    )

            def tree_d(src, K, dst):
                """am [128,K,N,D] bf16 -> dst a [128,K,N] via halving adds on
                innermost d. dst lands directly in logit [K,N] layout."""
                cur = src
                w = D
                with nc.allow_low_precision(reason="tree bf16 validated"):
                    while w > 2:
                        w //= 2
                        nxt = tmp_pool.tile(
                            [128, K, N, w], dt.bfloat16, tag=f"td{w}",
                            name=f"td{w}",
                        )
                        nc.vector.tensor_tensor(
                            nxt[:], cur[:, :, :, 0:w], cur[:, :, :, w : 2 * w],
                            op=OP.add,
                        )
                        cur = nxt
                    nc.vector.tensor_tensor(
                        dst, cur[:, :, :, 0], cur[:, :, :, 1], op=OP.add
                    )

            def t_bc(tt, K):
                t3 = tt[:]
                return _ap(t3, [t3.ap[0], [D, K], [0, N], [1, D]])

            def routing_r0(bi, k0, k1):
                """agreement pass a0 = uh . t0 for chunk sub-range [k0,k1)."""
                K = k1 - k0
                uhd = uhd_all[bi]
                tt = t_all[bi]
                t3 = tt[:, k0:k1, :]
                am = tmp_pool.tile(
                    [128, K, N, D], dt.bfloat16, tag=f"am{K}", name=f"am{K}"
                )
                nc.vector.tensor_tensor(
                    am.rearrange("p a b c -> p (a b c)"),
                    uhd[:, k0:k1, :, :].rearrange("p a b c -> p (a b c)"),
                    _ap(t3, [t3.ap[0], [D, K], [0, N], [1, D]]),
                    op=OP.mult,
                )
                a0 = sm_pool.tile(
                    [128, K, N], dt.bfloat16, tag=f"a0_{bi}_{k0}",
                    name=f"a0_{bi}_{k0}",
                )
                tree_d(am, K, a0[:])
                return a0

            def routing_r0_fin(bi, a0s):
                """scalar chain for round 0 + b1 = alpha0 * a0."""
                chs = batches[bi]
                K = len(chs)
                tt = t_all[bi]
                bA, _ = b_all[bi]
                sq = sm_pool.tile([128, K, D], dt.bfloat16, tag="sqv", name="sqv")
                nc.vector.tensor_tensor(sq[:], tt[:], tt[:], op=OP.mult)
                q = smt(K, "q")
                nc.vector.tensor_reduce(q[:], sq[:], axis=AX.X, op=OP.add)
                den = smt(K, "den")
                nc.vector.tensor_single_scalar(
                    den[:], q[:], float(N * N), op=OP.add
                )
                rden = smt(K, "rden")
                nc.vector.reciprocal(rden[:], den[:])
                sm = sqrt_half(q[:], K)
                alpha = smt(K, "alpha")
                nc.vector.tensor_mul(alpha[:], sm, rden[:])
                # b1 = alpha_bc * a0  (alpha broadcast along n: inner step 0
                # -> 1x, but FD is only K*N)
                a_bc = _ap(alpha[:], [*alpha[:].ap, [0, N]])
                off = 0
                for a0 in a0s:
                    Ka = a0[:].shape[1]
                    al3 = alpha[:, off : off + Ka]
                    nc.vector.tensor_tensor(
                        bA[:, off : off + Ka, :],
                        a0[:],
                        _ap(al3, [*al3.ap, [0, N]]),
                        op=OP.mult,
                    )
                    off += Ka

            def routing_round(bi, r):
                """rounds 1..2: softmax-weighted sum + (r==1) agreement."""
                chs = batches[bi]
                K = len(chs)
                uhd, uhn = uhd_all[bi], uhn_all[bi]
                tt = t_all[bi]
                bA, bB = b_all[bi]
                bcur = bA if r == 1 else bB
                if r == 2:
                    # r2 logits can reach ~56; subtract the max so se^2
                    # stays in fp32 range. r1 logits are <~33, exp directly.
                    mx = smt(K, "mx", dt.bfloat16)
                    with nc.allow_low_precision(reason="bf16 logits"):
                        nc.vector.tensor_reduce(
                            mx[:], bcur[:], axis=AX.X, op=OP.max
                        )
                    bsub = sm_pool.tile(
                        [128, K, N], dt.bfloat16, tag="bsub", name="bsub"
                    )
                    nc.vector.tensor_tensor(
                        bsub[:],
                        bcur[:],
                        _ap(mx[:], [*mx[:].ap, [0, N]]),
                        op=OP.subtract,
                    )
                    esrc = bsub[:]
                else:
                    esrc = bcur[:]
                e = sm_pool.tile([128, K, N], dt.bfloat16, tag="esm", name="esm")
                nc.scalar.activation(e[:], esrc, AF.Exp)
                se = smt(K, "se")
                nc.vector.tensor_reduce(se[:], e[:], axis=AX.X, op=OP.add)
                # wm = uh_nmaj * e  (e broadcast along d: [0,D] outer,
                # [1,N] inner -> 2x mode, no materialization)
                e3 = e[:]
                wm = tmp_pool.tile(
                    [128, K, D, N], dt.bfloat16, tag="wm", name="wm"
                )
                nc.vector.tensor_tensor(
                    wm.rearrange("p a b c -> p (a b c)"),
                    uhn[:].rearrange("p a b c -> p (a b c)"),
                    _ap(e3, [e3.ap[0], [N, K], [0, D], [1, N]]),
                    op=OP.mult,
                )
                tree_n(wm, K, tt[:])
                # q = |t|^2, den = se^2 + q, rden = 1/den
                sq = sm_pool.tile([128, K, D], dt.bfloat16, tag="sqv", name="sqv")
                nc.vector.tensor_tensor(sq[:], tt[:], tt[:], op=OP.mult)
                q = smt(K, "q")
                nc.vector.tensor_reduce(q[:], sq[:], axis=AX.X, op=OP.add)
                se2 = smt(K, "se2")
                nc.vector.tensor_mul(se2[:], se[:], se[:])
                den = smt(K, "den")
                nc.vector.tensor_add(den[:], q[:], se2[:])
                rden = smt(K, "rden")
                nc.vector.reciprocal(rden[:], den[:])
                if r == 1:
                    sm = sqrt_half(q[:], K)
                    alpha = smt(K, "alpha")
                    nc.vector.tensor_mul(alpha[:], sm, rden[:])
                    am = tmp_pool.tile(
                        [128, K, N, D], dt.bfloat16, tag=f"am{K}", name=f"am{K}"
                    )
                    nc.vector.tensor_tensor(
                        am.rearrange("p a b c -> p (a b c)"),
                        uhd[:].rearrange("p a b c -> p (a b c)"),
                        t_bc(tt, K),
                        op=OP.mult,
                    )
                    a1 = sm_pool.tile(
                        [128, K, N], dt.bfloat16, tag="a1", name="a1"
                    )
                    tree_d(am, K, a1[:])
                    badd = sm_pool.tile(
                        [128, K, N], dt.bfloat16, tag="badd", name="badd"
                    )
                    nc.vector.tensor_tensor(
                        badd[:],
                        a1[:],
                        _ap(alpha[:], [*alpha[:].ap, [0, N]]),
                        op=OP.mult,
                    )
                    with nc.allow_low_precision(reason="bf16 logits"):
                        nc.vector.tensor_tensor(
                            bB[:], bA[:], badd[:], op=OP.add
                        )
                else:
                    nc.vector.tensor_mul(
                        out_sb[:, chs[0] : chs[0] + K], q[:], rden[:]
                    )
                    nc.sync.dma_start(
                        out=out_ap.rearrange("(c p) -> p c", p=128)[
                            :, chs[0] : chs[0] + K
                        ],
                        in_=out_sb[:, chs[0] : chs[0] + K],
                    )

            # PE p-state warmup while super 0 streams in
            psw = psw_pool.tile([128, SUP], dt.float32)
            for wi in range(24):
                nc.tensor.matmul(
                    psw[:],
                    warm_sb[:, 0, 0:128],
                    warm_sb[:, 1, :],
                    start=(wi == 0),
                    stop=(wi == 23),
                )

            def mm_super(sp):
                xs = xs_pool.tile([128, NH, SUP], dt.float8e4)
                nc.sync.dma_start(out=xs[:], in_=xt_ap[sp])
                if sp == 0:
                    issue_params()
                psp = psp_pool.tile([128, SUP], dt.float32)
                # Linear bias rides as a K=1 bf16 matmul against ones
                ones_bc = _ap(
                    bias_sb[:, 128:256],
                    [bias_sb[:, 128:256].ap[0], [0, CPS], [1, 128]],
                )
                nc.tensor.matmul(
                    psp[:], bias_sb[:, 0:128], ones_bc, start=True, stop=False
                )
                for hp in range(NH // 2):
                    nc.tensor.matmul(
                        psp[:],
                        w_sb[:, 2 * hp : 2 * hp + 2, :],
                        xs[:, 2 * hp : 2 * hp + 2, :],
                        start=False,
                        stop=(hp == NH // 2 - 1),
                        perf_mode=PM.DoubleRow,
                    )
                primt = primt_pool.tile([128, SUP], dt.bfloat16)
                nc.scalar.copy(primt[:], psp[:])

                for c in range(CPS):
                    s = sp * CPS + c
                    bi = chunk_to_batch[s]
                    k = s - batches[bi][0]
                    lhsT = primt[:, c * 128 : (c + 1) * 128]
                    psu_d = psu_pool.tile([128, N * D], dt.float32)
                    nc.tensor.matmul(
                        psu_d[:], lhsT, capsd_sb[:], start=True, stop=True
                    )
                    psu_n = psu_pool.tile([128, D * N], dt.float32)
                    nc.tensor.matmul(
                        psu_n[:], lhsT, capsn_sb[:], start=True, stop=True
                    )
                    pst = pst_pool.tile([128, CAP_DIM], dt.float32)
                    nc.tensor.matmul(
                        pst[:], lhsT, capsum_sb[:], start=True, stop=True
                    )
                    nc.scalar.copy(
                        uhd_all[bi][:, k, :, :],
                        psu_d.rearrange("p (n d) -> p n d", n=N),
                    )
                    nc.scalar.copy(
                        uhn_all[bi][:, k, :, :],
                        psu_n.rearrange("p (d n) -> p d n", d=D),
                    )
                    nc.scalar.copy(t_all[bi][:, k, :], pst[:])

            # ---- emission schedule (two groups of 8 chunks = 2 supers each)
            # supers 0,1 -> group 0; r0 per-super as data arrives; then
            # interleave group-0 rounds with supers 2,3 so ACT copies never
            # queue behind chain-critical exps.
            mm_super(0)
            a0_g0_first = routing_r0(0, 0, 4)
            mm_super(1)
            a0_g0_second = routing_r0(0, 4, 8)
            routing_r0_fin(0, [a0_g0_first, a0_g0_second])
            mm_super(2)
            routing_round(0, 1)
            mm_super(3)
            routing_round(0, 2)
            a1_g1_first = routing_r0(1, 0, 4)
            a1_g1_second = routing_r0(1, 4, 8)
            routing_r0_fin(1, [a1_g1_first, a1_g1_second])
            routing_round(1, 1)
            routing_round(1, 2)

    nc.compile()
    return nc


def _prep_params(W, b_lin, out_caps, hidden=HIDDEN):
    NH = hidden // 128
    w_f = np.ascontiguousarray(
        (W.astype(np.float32) * W_SCALE)
        .reshape(NH, 128, NUM_CAPS * CAP_DIM)
        .transpose(1, 0, 2)
    ).astype(FP8)
    # full_caps[ic, n, d]: block-diagonal per (o,i): rows i*16..i*16+15
    full_caps = np.zeros((128, N_ROUTE, CAP_DIM), np.float32)
    for o in range(NUM_OBJ):
        for i in range(NUM_CAPS):
            full_caps[
                i * CAP_DIM : (i + 1) * CAP_DIM, o * NUM_CAPS + i, :
            ] = out_caps[o, i]
    full_caps /= W_SCALE
    capsd = np.ascontiguousarray(full_caps.reshape(128, -1)).astype(BF16)
    capsn = np.ascontiguousarray(
        full_caps.transpose(0, 2, 1).reshape(128, -1)
    ).astype(BF16)
    capsum = np.ascontiguousarray(full_caps.sum(1)).astype(BF16)
    bias_row = np.concatenate(
        [
            b_lin.astype(np.float32).reshape(1, 128) * W_SCALE,
            np.ones((1, 128), np.float32),
        ],
        axis=1,
    ).astype(BF16)
    return w_f, capsd, capsn, capsum, bias_row


_NC_CACHE = {}


def kernel(x, W, b_lin, out_caps):
    global LAST_EXEC_TIME_NS
    from concourse.bass_utils import run_bass_kernel_spmd

    x = np.asarray(x)
    W = np.asarray(W)
    b_lin = np.asarray(b_lin)
    out_caps = np.asarray(out_caps)
    bsz, hidden = x.shape
    b_sh = bsz // N_CORES
    NH = hidden // 128
    SUP = 512
    NSUP = b_sh // SUP

    key = (hidden, b_sh)
    if key not in _NC_CACHE:
        _NC_CACHE[key] = build_bass(hidden=hidden, b_sh=b_sh)
    nc = _NC_CACHE[key]

    w_f, capsd, capsn, capsum, bias_row = _prep_params(W, b_lin, out_caps, hidden)

    in_maps = []
    for i in range(N_CORES):
        shard = x[i * b_sh : (i + 1) * b_sh]
        # [sp, p, hp, b]: one fully-contiguous 2MB DMA per super,
        # 16KB contiguous per partition
        xt = np.ascontiguousarray(
            shard.reshape(NSUP, SUP, NH, 128).transpose(0, 3, 2, 1)
        ).astype(FP8)
        in_maps.append(
            {
                "xt": xt,
                "w": w_f,
                "capsd": capsd,
                "capsn": capsn,
                "capsum": capsum,
                "bias": bias_row,
            }
        )

    res = run_bass_kernel_spmd(
        nc,
        in_maps,
        core_ids=list(range(N_CORES)),
        trace=bool(int(os.environ.get("BASS_TRACE", "0") or "0")),
    )
    LAST_EXEC_TIME_NS = res.exec_time_ns
    return np.concatenate([res.results[i]["out"] for i in range(N_CORES)])


# revision 15
# speedup vs baseline: 1.3538x; 1.0667x over previous
"""CapsuleRewardHead Trainium2 kernel (8-core data parallel), v2.

Math (per batch row b):
    primary = x @ W + b_lin                    [B, 128]  (128 = 8 caps x 16 dim)
    u_hat[b,o,i,j] = sum_c primary[b,i,c] * out_caps[o,i,c,j]
    3 rounds of dynamic routing over N=32 capsule pairs (o,i), D=16
    out[b] = |squash(s_final)|

Device strategy per core (2048 batch rows):
  - host: quantize x shard to fp8 e4m3, laid out [sp][128 part][hp][b] so each
    super is ONE contiguous 2MB DMA (16KB/partition) -> ~6us super latency,
    full 16-SDMA-engine spread, pipelined with MM1.
  - MM1 (PE): DoubleRow fp8 matmuls contract h-chunk pairs into PSUM:
    primaryT[ic, b] per 512-col super; linear bias rides as a K=1 bf16 matmul.
  - MM2 (PE): per 128-row chunk, TWO matmuls against differently-ordered
    block-diagonal caps constants give u_hat in both [K,N,D] (d-inner) and
    [K,D,N] (n-inner) layouts, plus a capsum matmul for round-0's t0.
  - routing: all elementwise on DVE with DIRECT broadcast reads (inner-step-1
    APs hit 2x mode on HW; verified by microbench — no erep/trep
    materialization, no GPSIMD which contends with DVE for the SBUF port).
    n-trees run on the n-inner copy, d-trees on the d-inner copy so every
    tree level is a 2x-mode halving add and the agreement lands directly in
    the [K,N] logit layout. sqrt via bit-trick seed; unnormalized
    accumulators (q = |t|^2, se = sum e).
  - emission order interleaves MM2 chunk blocks with group-0 rounds so ACT
    psum->sbuf copies never queue behind chain-critical exps.
"""

import os

import numpy as np
import ml_dtypes

B = 16384
HIDDEN = 4096
NUM_OBJ = 4
NUM_CAPS = 8
CAP_DIM = 16
N_ROUTE = 32  # NUM_OBJ * NUM_CAPS
N_CORES = 8

LAST_EXEC_TIME_NS = None  # set after each run when BASS_TRACE=1

BF16 = ml_dtypes.bfloat16
FP8 = ml_dtypes.float8_e4m3
W_SCALE = 1024.0
SQRT_MAGIC = 0x1FBD1DF5


def _ap(ap, dims):
    import concourse.bass as bass

    return bass.AP(tensor=ap.tensor, offset=ap.offset, ap=dims)


def build_bass(hidden=HIDDEN, b_sh=B // N_CORES, batch_plan=(8, 8)):
    import concourse.tile as tile
    from concourse import bacc, mybir

    NH = hidden // 128
    NCH = b_sh // 128  # 128-row chunks
    SUP = 512
    NSUP = b_sh // SUP
    CPS = SUP // 128
    assert sum(batch_plan) == NCH
    N, D = N_ROUTE, CAP_DIM
    dt = mybir.dt
    AX = mybir.AxisListType
    OP = mybir.AluOpType
    AF = mybir.ActivationFunctionType
    PM = mybir.MatmulPerfMode

    batches = []
    pos = 0
    for k in batch_plan:
        batches.append(list(range(pos, pos + k)))
        pos += k
    chunk_to_batch = {}
    for bi, chs in enumerate(batches):
        for ch in chs:
            chunk_to_batch[ch] = bi

    nc = bacc.Bacc("TRN2", target_bir_lowering=False, debug=False, num_devices=N_CORES)

    NPC = 4  # DMA pieces per super
    HQ = NH // NPC
    xt_ap = nc.dram_tensor(
        "xt", [NSUP, NPC, 128, HQ, SUP], dt.float8e4, kind="ExternalInput"
    ).ap()
    w_ap = nc.dram_tensor("w", [128, NH, 128], dt.float8e4, kind="ExternalInput").ap()
    capsd_ap = nc.dram_tensor(
        "capsd", [128, N * D], dt.bfloat16, kind="ExternalInput"
    ).ap()
    capsn_ap = nc.dram_tensor(
        "capsn", [128, D * N], dt.bfloat16, kind="ExternalInput"
    ).ap()
    capsum_ap = nc.dram_tensor(
        "capsum", [128, CAP_DIM], dt.bfloat16, kind="ExternalInput"
    ).ap()
    bias_ap = nc.dram_tensor("bias", [1, 256], dt.bfloat16, kind="ExternalInput").ap()
    out_ap = nc.dram_tensor("out", [b_sh], dt.float32, kind="ExternalOutput").ap()

    with tile.TileContext(nc) as tc:
        with (
            tc.tile_pool(name="singles", bufs=1) as singles,
            tc.tile_pool(name="xs", bufs=NSUP * NPC) as xs_pool,
            tc.tile_pool(name="primt", bufs=2) as primt_pool,
            tc.tile_pool(name="batch", bufs=1) as bpool,
            tc.tile_pool(name="tmp", bufs=2) as tmp_pool,
            tc.tile_pool(name="sm", bufs=8) as sm_pool,
            tc.tile_pool(name="psum_p", bufs=2, space="PSUM") as psp_pool,
            tc.tile_pool(name="psum_u", bufs=3, space="PSUM") as psu_pool,
            tc.tile_pool(name="psum_t", bufs=2, space="PSUM") as pst_pool,
            tc.tile_pool(name="psum_w", bufs=1, space="PSUM") as psw_pool,
        ):
            w_sb = singles.tile([128, NH, 128], dt.float8e4)
            capsd_sb = singles.tile([128, N * D], dt.bfloat16)
            capsn_sb = singles.tile([128, D * N], dt.bfloat16)
            capsum_sb = singles.tile([128, CAP_DIM], dt.bfloat16)
            bias_sb = singles.tile([1, 256], dt.bfloat16)

            def issue_params():
                # qAct HWDGE ring so params don't delay the x stream on qSP
                nc.scalar.dma_start(out=w_sb[:], in_=w_ap[:, :, :])
                nc.scalar.dma_start(out=capsd_sb[:], in_=capsd_ap[:, :])
                nc.scalar.dma_start(out=capsn_sb[:], in_=capsn_ap[:, :])
                nc.scalar.dma_start(out=capsum_sb[:], in_=capsum_ap[:, :])
                nc.scalar.dma_start(out=bias_sb[:], in_=bias_ap[:, :])

            magic_sb = singles.tile([128, 1], dt.uint32)
            nc.vector.memset(magic_sb[:], SQRT_MAGIC)
            out_sb = singles.tile([128, NCH], dt.float32)
            warm_sb = singles.tile([128, 2, SUP], dt.float8e4)
            nc.vector.memset(warm_sb.rearrange("p a b -> p (a b)"), 0)

            uhd_all, uhn_all, t_all, b_all = {}, {}, {}, {}
            for bi, chs in enumerate(batches):
                K = len(chs)
                uhd_all[bi] = bpool.tile(
                    [128, K, N, D], dt.bfloat16, tag=f"uhd{bi}", name=f"uhd{bi}"
                )
                uhn_all[bi] = bpool.tile(
                    [128, K, D, N], dt.bfloat16, tag=f"uhn{bi}", name=f"uhn{bi}"
                )
                t_all[bi] = bpool.tile(
                    [128, K, D], dt.bfloat16, tag=f"t{bi}", name=f"t{bi}"
                )
                # two logit buffers: the r1 update writes out-of-place
                # (in-place DVE ops run ~4x slower), bf16 for 2x mode
                b_all[bi] = (
                    bpool.tile([128, K, N], dt.bfloat16, tag=f"b{bi}a",
                               name=f"b{bi}a"),
                    bpool.tile([128, K, N], dt.bfloat16, tag=f"b{bi}b",
                               name=f"b{bi}b"),
                )

            def smt(K, tag, dtype=dt.float32):
                return sm_pool.tile([128, K], dtype, tag=tag, name=tag)

            def sqrt_half(q, K):
                """bit-trick sqrt seed; error washes out through squash."""
                qu = q.bitcast(dt.uint32)
                s1 = smt(K, "sq1", dt.uint32)
                nc.vector.tensor_single_scalar(
                    s1[:], qu, 1, op=OP.logical_shift_right
                )
                s2 = smt(K, "sq2", dt.uint32)
                nc.vector.tensor_tensor(
                    s2[:],
                    s1[:],
                    _ap(magic_sb[:], [magic_sb[:].ap[0], [0, K]]),
                    op=OP.add,
                )
                return s2.bitcast(dt.float32)  # ~3.5% sqrt approx (validated)

            def tree_n(src, K, dst):
                """wm [128,K,D,N] bf16 -> dst t [128,K,D] via halving adds on
                innermost n (every level inner step 1 -> 2x mode)."""
                cur = src
                w = N
                with nc.allow_low_precision(reason="tree bf16 validated"):
                    while w > 2:
                        w //= 2
                        nxt = tmp_pool.tile(
                            [128, K, D, w], dt.bfloat16, tag=f"tn{w}",
                            name=f"tn{w}",
                        )
                        nc.vector.tensor_tensor(
                            nxt[:], cur[:, :, :, 0:w], cur[:, :, :, w : 2 * w],
                            op=OP.add,
                        )
                        cur = nxt
                    nc.vector.tensor_tensor(
                        dst, cur[:, :, :, 0], cur[:, :, :, 1], op=OP.add
                    )

            def tree_d(src, K, dst):
                """am [128,K,N,D] bf16 -> dst a [128,K,N] via halving adds on
                innermost d. dst lands directly in logit [K,N] layout."""
                cur = src
                w = D
                with nc.allow_low_precision(reason="tree bf16 validated"):
                    while w > 2:
                        w //= 2
                        nxt = tmp_pool.tile(
                            [128, K, N, w], dt.bfloat16, tag=f"td{w}",
                            name=f"td{w}",
                        )
                        nc.vector.tensor_tensor(
                            nxt[:], cur[:, :, :, 0:w], cur[:, :, :, w : 2 * w],
                            op=OP.add,
                        )
                        cur = nxt
                    nc.vector.tensor_tensor(
                        dst, cur[:, :, :, 0], cur[:, :, :, 1], op=OP.add
                    )

            def t_bc(tt, K):
                t3 = tt[:]
                return _ap(t3, [t3.ap[0], [D, K], [0, N], [1, D]])

            def routing_r0(bi, k0, k1):
                """agreement pass a0 = uh . t0 for chunk sub-range [k0,k1)."""
                K = k1 - k0
                uhd = uhd_all[bi]
                tt = t_all[bi]
                t3 = tt[:, k0:k1, :]
                am = tmp_pool.tile(
                    [128, K, N, D], dt.bfloat16, tag=f"am{K}", name=f"am{K}"
                )
                nc.vector.tensor_tensor(
                    am.rearrange("p a b c -> p (a b c)"),
                    uhd[:, k0:k1, :, :].rearrange("p a b c -> p (a b c)"),
                    _ap(t3, [t3.ap[0], [D, K], [0, N], [1, D]]),
                    op=OP.mult,
                )
                a0 = sm_pool.tile(
                    [128, K, N], dt.bfloat16, tag=f"a0_{bi}_{k0}",
                    name=f"a0_{bi}_{k0}",
                )
                tree_d(am, K, a0[:])
                return a0

            def routing_r0_fin(bi, a0s):
                """scalar chain for round 0 + b1 = alpha0 * a0."""
                chs = batches[bi]
                K = len(chs)
                tt = t_all[bi]
                bA, _ = b_all[bi]
                sq = sm_pool.tile([128, K, D], dt.bfloat16, tag="sqv", name="sqv")
                nc.vector.tensor_tensor(sq[:], tt[:], tt[:], op=OP.mult)
                q = smt(K, "q")
                nc.vector.tensor_reduce(q[:], sq[:], axis=AX.X, op=OP.add)
                den = smt(K, "den")
                nc.vector.tensor_single_scalar(
                    den[:], q[:], float(N * N), op=OP.add
                )
                rden = smt(K, "rden")
                nc.vector.reciprocal(rden[:], den[:])
                sm = sqrt_half(q[:], K)
                alpha = smt(K, "alpha")
                nc.vector.tensor_mul(alpha[:], sm, rden[:])
                # b1 = alpha_bc * a0  (alpha broadcast along n: inner step 0
                # -> 1x, but FD is only K*N)
                a_bc = _ap(alpha[:], [*alpha[:].ap, [0, N]])
                off = 0
                for a0 in a0s:
                    Ka = a0[:].shape[1]
                    al3 = alpha[:, off : off + Ka]
                    nc.vector.tensor_tensor(
                        bA[:, off : off + Ka, :],
                        a0[:],
                        _ap(al3, [*al3.ap, [0, N]]),
                        op=OP.mult,
                    )
                    off += Ka

            def routing_round(bi, r):
                """rounds 1..2: softmax-weighted sum + (r==1) agreement."""
                chs = batches[bi]
                K = len(chs)
                uhd, uhn = uhd_all[bi], uhn_all[bi]
                tt = t_all[bi]
                bA, bB = b_all[bi]
                bcur = bA if r == 1 else bB
                if r == 2:
                    # r2 logits can reach ~56; subtract the max so se^2
                    # stays in fp32 range. r1 logits are <~33, exp directly.
                    mx = smt(K, "mx", dt.bfloat16)
                    with nc.allow_low_precision(reason="bf16 logits"):
                        nc.vector.tensor_reduce(
                            mx[:], bcur[:], axis=AX.X, op=OP.max
                        )
                    bsub = sm_pool.tile(
                        [128, K, N], dt.bfloat16, tag="bsub", name="bsub"
                    )
                    nc.vector.tensor_tensor(
                        bsub[:],
                        bcur[:],
                        _ap(mx[:], [*mx[:].ap, [0, N]]),
                        op=OP.subtract,
                    )
                    esrc = bsub[:]
                else:
                    esrc = bcur[:]
                e = sm_pool.tile([128, K, N], dt.bfloat16, tag="esm", name="esm")
                nc.scalar.activation(e[:], esrc, AF.Exp)
                se = smt(K, "se")
                nc.vector.tensor_reduce(se[:], e[:], axis=AX.X, op=OP.add)
                # wm = uh_nmaj * e  (e broadcast along d: [0,D] outer,
                # [1,N] inner -> 2x mode, no materialization)
                e3 = e[:]
                wm = tmp_pool.tile(
                    [128, K, D, N], dt.bfloat16, tag="wm", name="wm"
                )
                nc.vector.tensor_tensor(
                    wm.rearrange("p a b c -> p (a b c)"),
                    uhn[:].rearrange("p a b c -> p (a b c)"),
                    _ap(e3, [e3.ap[0], [N, K], [0, D], [1, N]]),
                    op=OP.mult,
                )
                tree_n(wm, K, tt[:])
                # q = |t|^2, den = se^2 + q, rden = 1/den
                sq = sm_pool.tile([128, K, D], dt.bfloat16, tag="sqv", name="sqv")
                nc.vector.tensor_tensor(sq[:], tt[:], tt[:], op=OP.mult)
                q = smt(K, "q")
                nc.vector.tensor_reduce(q[:], sq[:], axis=AX.X, op=OP.add)
                se2 = smt(K, "se2")
                nc.vector.tensor_mul(se2[:], se[:], se[:])
                den = smt(K, "den")
                nc.vector.tensor_add(den[:], q[:], se2[:])
                rden = smt(K, "rden")
                nc.vector.reciprocal(rden[:], den[:])
                if r == 1:
                    sm = sqrt_half(q[:], K)
                    alpha = smt(K, "alpha")
                    nc.vector.tensor_mul(alpha[:], sm, rden[:])
                    am = tmp_pool.tile(
                        [128, K, N, D], dt.bfloat16, tag=f"am{K}", name=f"am{K}"
                    )
                    nc.vector.tensor_tensor(
                        am.rearrange("p a b c -> p (a b c)"),
                        uhd[:].rearrange("p a b c -> p (a b c)"),
                        t_bc(tt, K),
                        op=OP.mult,
                    )
                    a1 = sm_pool.tile(
                        [128, K, N], dt.bfloat16, tag="a1", name="a1"
                    )
                    tree_d(am, K, a1[:])
                    badd = sm_pool.tile(
                        [128, K, N], dt.bfloat16, tag="badd", name="badd"
                    )
                    nc.vector.tensor_tensor(
                        badd[:],
                        a1[:],
                        _ap(alpha[:], [*alpha[:].ap, [0, N]]),
                        op=OP.mult,
                    )
                    with nc.allow_low_precision(reason="bf16 logits"):
                        nc.vector.tensor_tensor(
                            bB[:], bA[:], badd[:], op=OP.add
                        )
                else:
                    nc.vector.tensor_mul(
                        out_sb[:, chs[0] : chs[0] + K], q[:], rden[:]
                    )
                    nc.sync.dma_start(
                        out=out_ap.rearrange("(c p) -> p c", p=128)[
                            :, chs[0] : chs[0] + K
                        ],
                        in_=out_sb[:, chs[0] : chs[0] + K],
                    )

            # ---- issue the whole x stream upfront: 16 piece-DMAs on the
            # qSP ring pipeline back-to-back; params ride the qAct ring.
            xs_tiles = {}
            for sp in range(NSUP):
                for pc in range(NPC):
                    xs = xs_pool.tile([128, HQ, SUP], dt.float8e4)
                    nc.sync.dma_start(out=xs[:], in_=xt_ap[sp, pc])
                    xs_tiles[(sp, pc)] = xs
                    if sp == 0 and pc == 0:
                        issue_params()

            # PE p-state warmup sized to end as piece (0,0) lands
            psw = psw_pool.tile([128, SUP], dt.float32)
            for wi in range(8):
                nc.tensor.matmul(
                    psw[:],
                    warm_sb[:, 0, 0:128],
                    warm_sb[:, 1, :],
                    start=(wi == 0),
                    stop=(wi == 7),
                )

            primt_all = {}

            def mm1_super(sp):
                psp = psp_pool.tile([128, SUP], dt.float32)
                # Linear bias rides as a K=1 bf16 matmul against ones
                ones_bc = _ap(
                    bias_sb[:, 128:256],
                    [bias_sb[:, 128:256].ap[0], [0, CPS], [1, 128]],
                )
                nc.tensor.matmul(
                    psp[:], bias_sb[:, 0:128], ones_bc, start=True, stop=False
                )
                for pc in range(NPC):
                    xs = xs_tiles[(sp, pc)]
                    for hp in range(HQ // 2):
                        h = pc * HQ + 2 * hp
                        nc.tensor.matmul(
                            psp[:],
                            w_sb[:, h : h + 2, :],
                            xs[:, 2 * hp : 2 * hp + 2, :],
                            start=False,
                            stop=(pc == NPC - 1 and hp == HQ // 2 - 1),
                            perf_mode=PM.DoubleRow,
                        )
                primt = primt_pool.tile([128, SUP], dt.bfloat16)
                nc.scalar.copy(primt[:], psp[:])
                primt_all[sp] = primt

            def mm2_d(sp):
                """uh_dmaj + t0 for the 4 chunks of super sp (feeds r0)."""
                for c in range(CPS):
                    s = sp * CPS + c
                    bi = chunk_to_batch[s]
                    k = s - batches[bi][0]
                    lhsT = primt_all[sp][:, c * 128 : (c + 1) * 128]
                    psu_d = psu_pool.tile(
                        [128, N * D], dt.float32, tag="psu", name="psu_d"
                    )
                    nc.tensor.matmul(
                        psu_d[:], lhsT, capsd_sb[:], start=True, stop=True
                    )
                    pst = pst_pool.tile([128, CAP_DIM], dt.float32)
                    nc.tensor.matmul(
                        pst[:], lhsT, capsum_sb[:], start=True, stop=True
                    )
                    nc.scalar.copy(
                        uhd_all[bi][:, k, :, :],
                        psu_d.rearrange("p (n d) -> p n d", n=N),
                    )
                    nc.scalar.copy(t_all[bi][:, k, :], pst[:])

            def mm2_n(sp):
                """uh_nmaj for the 4 chunks of super sp (feeds r1/r2)."""
                for c in range(CPS):
                    s = sp * CPS + c
                    bi = chunk_to_batch[s]
                    k = s - batches[bi][0]
                    lhsT = primt_all[sp][:, c * 128 : (c + 1) * 128]
                    psu_n = psu_pool.tile(
                        [128, D * N], dt.float32, tag="psu", name="psu_n"
                    )
                    nc.tensor.matmul(
                        psu_n[:], lhsT, capsn_sb[:], start=True, stop=True
                    )
                    nc.scalar.copy(
                        uhn_all[bi][:, k, :, :],
                        psu_n.rearrange("p (d n) -> p d n", d=D),
                    )

            # ---- emission schedule: two groups of 8 chunks (2 supers each),
            # ordered by data arrival so no engine FIFO entry blocks a
            # later-emitted but earlier-ready op.
            mm1_super(0)
            mm2_d(0)
            a0_g0_first = routing_r0(0, 0, 4)
            mm2_n(0)
            mm1_super(1)
            mm2_d(1)
            a0_g0_second = routing_r0(0, 4, 8)
            routing_r0_fin(0, [a0_g0_first, a0_g0_second])
            mm2_n(1)
            routing_round(0, 1)
            mm1_super(2)
            mm2_d(2)
            mm2_n(2)
            routing_round(0, 2)
            mm1_super(3)
            mm2_d(3)
            mm2_n(3)
            a1_g1_first = routing_r0(1, 0, 4)
            a1_g1_second = routing_r0(1, 4, 8)
            routing_r0_fin(1, [a1_g1_first, a1_g1_second])
            routing_round(1, 1)
            routing_round(1, 2)

    nc.compile()
    return nc


def _prep_params(W, b_lin, out_caps, hidden=HIDDEN):
    NH = hidden // 128
    w_f = np.ascontiguousarray(
        (W.astype(np.float32) * W_SCALE)
        .reshape(NH, 128, NUM_CAPS * CAP_DIM)
        .transpose(1, 0, 2)
    ).astype(FP8)
    # full_caps[ic, n, d]: block-diagonal per (o,i): rows i*16..i*16+15
    full_caps = np.zeros((128, N_ROUTE, CAP_DIM), np.float32)
    for o in range(NUM_OBJ):
        for i in range(NUM_CAPS):
            full_caps[
                i * CAP_DIM : (i + 1) * CAP_DIM, o * NUM_CAPS + i, :
            ] = out_caps[o, i]
    full_caps /= W_SCALE
    capsd = np.ascontiguousarray(full_caps.reshape(128, -1)).astype(BF16)
    capsn = np.ascontiguousarray(
        full_caps.transpose(0, 2, 1).reshape(128, -1)
    ).astype(BF16)
    capsum = np.ascontiguousarray(full_caps.sum(1)).astype(BF16)
    bias_row = np.concatenate(
        [
            b_lin.astype(np.float32).reshape(1, 128) * W_SCALE,
            np.ones((1, 128), np.float32),
        ],
        axis=1,
    ).astype(BF16)
    return w_f, capsd, capsn, capsum, bias_row


_NC_CACHE = {}


def kernel(x, W, b_lin, out_caps):
    global LAST_EXEC_TIME_NS
    from concourse.bass_utils import run_bass_kernel_spmd

    x = np.asarray(x)
    W = np.asarray(W)
    b_lin = np.asarray(b_lin)
    out_caps = np.asarray(out_caps)
    bsz, hidden = x.shape
    b_sh = bsz // N_CORES
    NH = hidden // 128
    SUP = 512
    NSUP = b_sh // SUP

    key = (hidden, b_sh)
    if key not in _NC_CACHE:
        _NC_CACHE[key] = build_bass(hidden=hidden, b_sh=b_sh)
    nc = _NC_CACHE[key]

    w_f, capsd, capsn, capsum, bias_row = _prep_params(W, b_lin, out_caps, hidden)

    in_maps = []
    for i in range(N_CORES):
        shard = x[i * b_sh : (i + 1) * b_sh]
        # [sp, pc, p, hq, b]: 4 contiguous 512KB piece-DMAs per super
        # (4KB contiguous per partition per piece)
        NPC = 4
        HQ = NH // NPC
        xt = np.ascontiguousarray(
            shard.reshape(NSUP, SUP, NPC, HQ, 128).transpose(0, 2, 4, 3, 1)
        ).astype(FP8)
        in_maps.append(
            {
                "xt": xt,
                "w": w_f,
                "capsd": capsd,
                "capsn": capsn,
                "capsum": capsum,
                "bias": bias_row,
            }
        )

    res = run_bass_kernel_spmd(
        nc,
        in_maps,
        core_ids=list(range(N_CORES)),
        trace=bool(int(os.environ.get("BASS_TRACE", "0") or "0")),
    )
    LAST_EXEC_TIME_NS = res.exec_time_ns
    return np.concatenate([res.results[i]["out"] for i in range(N_CORES)])
